# revision 75
# baseline (speedup 1.0000x reference)
"""Trainium2 Bass kernel for the 3-layer spiking neural network (DSNN).

Strategy
--------
Data-parallel over batch: 256 rows / 8 cores = 32 per core, weights
replicated, zero collectives. Inside each core:

  - Layer 2 has no reset, so mem2 = (sum_t w_t out1(t)) @ W2 exactly
    (closed-form alpha/beta decay weights): one small final matmul,
    split into two PSUM-accumulated halves so most of it runs during
    the pipeline-drain iterations.
  - The layer-1 synapse recurrence is folded into the matmul operand:
    mm1's moving tensor is the spike TRACE  strace_t = a*strace_{t-1} + s0_t,
    so  y1_t = strace_t @ W1  exactly (linearity) - no separate y1
    state, no per-step AXPY. The trace is produced by one fused DVE op
    per step (SNN_TRACE) straight into the fp16 matmul-operand slot.
  - Timestep-blocked matmuls (Tb=16 -> 512 moving columns). All
    weights, spikes and the trace run in fp16: halves LDWEIGHTS to
    ~135ns (the f32r LDW at 242ns was the PE rate limiter), streams
    at 1 cyc/row at any FD, and drops the f32r W0 tile + its DMA.
    Measured rel-l2 1.295e-2 vs the 2e-2 gate; the host-side
    bit-accurate simulator predicted 1.2946e-2 (it has matched HW to
    ~1e-4 on every variant tried).
  - Membrane recurrences of layer 0 (block k) and layer 1 (block k-2)
    run as ONE fused custom DVE op per step (SNN_RESET on [128,512],
    t-major contiguous drive slots [h0_t | h1_t] filled directly by
    the ScalarE PSUM drains of both matmuls): state is the negated
    membrane; 0.0 encodes "spiked". Layer-1 trails two blocks so its
    drive h1 = trace @ W1 is ready. Spike * w_t accumulation for the
    collapsed layer 2 is one more fused op (SNN_ABAR).
  - Membrane state ping-pongs between two tiles (nmA/nmB) so readers
    never stall the serial reset chain.

Per main step the DVE runs exactly TWO fused ops: SNN_RESET [128,512]
(both layers' membranes) and SNN_TA [128,2,256] (a SubIdx-paged op
whose page 0 is the trace EMA and page 1 the weighted-spike abar
accumulation, riding the 512-wide fp16 slots; ragged block boundaries
hand the running abar off to an fp32 tile). Measured step period
~1.33us (down from 2.65us/step in the original); PE runs 3072 fp16
moving columns well under that; ScalarE drains PSUM into the drive
slots. Measured: 190.0us (baseline 273.6us).
"""

import numpy as np

ALPHA = 0.9
BETA = 0.85
THR = 1.0
T = 99            # timesteps actually simulated (t = 1..99 of 100)
BCORE = 32        # batch per core
NCORES = 8
TBM = 16          # main block size (Nk = 512 moving cols)
BLOCK_SIZES = [16, 16, 16, 16, 16, 16, 3]
assert sum(BLOCK_SIZES) == T
NB = len(BLOCK_SIZES)
TSTART = [sum(BLOCK_SIZES[:i]) for i in range(NB)]

_CACHE = {}


def _register_custom_ops():
    """SNN_RESET: m = in0*s0 + in1; out = m>s1 ? 0 : -m   (negated membrane;
    0.0 encodes "spiked").
    SNN_TRACE: out = in1*s0 + (in0 == 0)                  (spike trace EMA).
    SNN_ABAR:  out = (in0 == 0) ? in1 + s0 : in1          (weighted spikes).
    """
    import concourse.dve_ops as dve_ops
    if "SNN_RESET" in dve_ops._SUB_OPCODE_FOR_NAME:
        return (next(o for o in dve_ops.OPS if o.name == "SNN_RESET"),
                next(o for o in dve_ops.OPS if o.name == "SNN_TRACE"),
                next(o for o in dve_ops.OPS if o.name == "SNN_ABAR"),
                next(o for o in dve_ops.OPS if o.name == "SNN_TA"))
    from concourse.dve_spec import (
        Spec, Src0, Src1, Zero, select, eq, lower, _has_src1, SubIdx)
    from concourse.dve_uop import DveOpSpec

    def make(name, spec, subdim=False):
        row = dve_ops._CUSTOM_DVE_ROW_BASE + len(dve_ops.OPS)
        assert row < 0x20
        dve_ops._SUB_OPCODE_FOR_NAME[name] = row
        shas = {}
        for ver in ("v3", "v4"):
            uops = lower(spec, ver=ver)
            shas[ver] = DveOpSpec(name=name, opcode=row, uops=uops,
                                  rd1_en=_has_src1(spec)).sha(ver)
        op = dve_ops.DveOp(name, spec, subdim=subdim, uops_sha=shas)
        dve_ops.OPS.append(op)
        dve_ops.CUSTOM_DVE_SPECS[name] = spec
        return op

    from concourse.dve_spec import C0, C1
    f32 = np.float32
    _m = Src0 * C0 + Src1
    reset = make("SNN_RESET", Spec(
        body=select(_m > C1, Zero, Zero - _m),
        reference=lambda in0, in1, s0, s1, imm2:
            np.where((in0 * f32(s0) + in1) > f32(s1),
                     f32(0.0), -(in0 * f32(s0) + in1)).astype(f32),
    ))
    trace = make("SNN_TRACE", Spec(
        body=Src1 * C0 + eq(Src0, Zero),
        reference=lambda in0, in1, s0, s1, imm2:
            (in1 * f32(s0) + (in0 == 0.0)).astype(f32),
    ))
    abar_op = make("SNN_ABAR", Spec(
        body=select(eq(Src0, Zero), Src1 + C0, Src1),
        reference=lambda in0, in1, s0, s1, imm2:
            np.where(in0 == 0.0, in1 + f32(s0), in1).astype(f32),
    ))

    def _ta_ref(in0, in1, s0, s1, imm2):
        # in0/in1/out: [P, 2, N] pages; page 0 = trace, page 1 = abar
        sp = (in0 == 0.0).astype(f32)
        out = np.empty_like(in1, dtype=f32)
        out[:, 0] = in1[:, 0] * f32(s0) + sp[:, 0]
        out[:, 1] = np.where(in0[:, 1] == 0.0, in1[:, 1] + f32(s1), in1[:, 1])
        return out

    _sp = eq(Src0, Zero)
    ta_op = make("SNN_TA", Spec(
        body=select(SubIdx, Src1 + _sp * C1, Src1 * C0 + _sp),
        reference=_ta_ref,
    ), subdim=True)
    return reset, trace, abar_op, ta_op


def _round_m11(x):
    # hw float32r = e8m11, round-to-nearest on the 12 dropped bits
    xi = np.ascontiguousarray(np.asarray(x, np.float32)).view(np.uint32).astype(np.uint64)
    bias = np.uint64(0x7FF) + ((xi >> np.uint64(12)) & np.uint64(1))
    return ((xi + bias) & np.uint64(0xFFFFF000)).astype(np.uint32).view(np.float32)


def _decay_weights():
    # w_j = sum_{k=0}^{T-1-j} BETA^(T-1-j-k) * ALPHA^k
    w = np.zeros(T, np.float64)
    for j in range(T):
        n = T - 1 - j
        k = np.arange(n + 1)
        w[j] = np.sum(BETA ** (n - k) * (ALPHA ** k))
    return w.astype(np.float32)


def build_program():
    if "nc" in _CACHE:
        return _CACHE["nc"]
    import concourse.bacc as bacc
    import concourse.mybir as mybir
    import concourse.tile as tile

    f32 = mybir.dt.float32
    f32r = mybir.dt.float32r
    f16 = mybir.dt.float16
    A = mybir.AluOpType
    Act = mybir.ActivationFunctionType

    OP_RESET, OP_TRACE, OP_ABAR, OP_TA = _register_custom_ops()
    W = _decay_weights()

    nc = bacc.Bacc("TRN2", target_bir_lowering=False, debug=False,
                   enable_asserts=False, num_devices=NCORES)

    RT = nc.dram_tensor("RT", [512, T * BCORE], f32, kind="ExternalInput").ap()
    xT = nc.dram_tensor("xT", [512, BCORE], f32, kind="ExternalInput").ap()
    W0h = nc.dram_tensor("W0h", [512, 1024], f16, kind="ExternalInput").ap()
    W1d = nc.dram_tensor("W1d", [1024, 1024], f16, kind="ExternalInput").ap()
    W2d = nc.dram_tensor("W2d", [1024, 512], f16, kind="ExternalInput").ap()
    b0d = nc.dram_tensor("b0d", [128, 8], f32, kind="ExternalInput").ap()
    outd = nc.dram_tensor("out", [BCORE, 512], f32, kind="ExternalOutput").ap()

    with tile.TileContext(nc) as tc:
        with (
            tc.tile_pool(name="const", bufs=1) as cpool,
            tc.tile_pool(name="rt", bufs=3) as rt_pool,
            tc.tile_pool(name="sblk", bufs=2) as s_pool,
            tc.tile_pool(name="s0p", bufs=2) as s0_pool,
            tc.tile_pool(name="drv", bufs=2) as drv_pool,
            tc.tile_pool(name="ps", bufs=4, space="PSUM") as ps_pool,
        ):
            # ---- constants ----
            w0h_sb = cpool.tile([128, 4 * 1024], f16, tag="w0h")
            w1_sb = cpool.tile([128, 8 * 1024], f16, tag="w1")
            b0_sb = cpool.tile([128, 8], f32, tag="b0")
            xt_sb = cpool.tile([128, 4 * BCORE], f32, tag="xt")

            nc.sync.dma_start(
                out=xt_sb[:].rearrange("p (c b) -> p c b", c=4),
                in_=xT.rearrange("(c p) b -> p c b", p=128))
            nc.sync.dma_start(out=b0_sb[:], in_=b0d)

            # ---- state ----
            # negm ping-pong: [0:256) = layer-0 negm (c,b), [256:512) = layer-1
            nmA = cpool.tile([128, 512], f32, tag="nmA")
            nmB = cpool.tile([128, 512], f32, tag="nmB")
            abar = cpool.tile([128, 256], f32, tag="abar")
            abar2 = cpool.tile([128, 256], f32, tag="abar2")
            for st in (nmA, nmB, abar, abar2):
                nc.vector.memset(st[:], 0.0)
            nm = [nmA, nmB]
            gstep = [0]

            rt4 = RT.rearrange("(c p) n -> p c n", p=128)
            rt_t, sblk_t, s0_t, drv_t = {}, {}, {}, {}

            def stage_dma_rt(k):
                Tb = BLOCK_SIZES[k]
                Nk = Tb * BCORE
                rt = rt_pool.tile([128, 4 * TBM * BCORE], f32, tag="rt")
                for c in range(4):
                    nc.sync.dma_start(
                        out=rt[:, c * Nk:(c + 1) * Nk],
                        in_=rt4[:, c, TSTART[k] * BCORE: TSTART[k] * BCORE + Nk])
                rt_t[k] = rt

            def stage_sg(k):
                # spike-gen: compare x (broadcast over t) against rt.
                # Output dtype matches the W0 flavor mm0 will use:
                # f32r for early blocks, fp16 for the small late blocks.
                Tb = BLOCK_SIZES[k]
                Nk = Tb * BCORE
                rt = rt_t.pop(k)
                sblk = s_pool.tile([128, 4 * TBM * BCORE], f16, tag="sblk")
                xc = (xt_sb[:].rearrange("p (c b) -> p c b", c=4)
                      .unsqueeze(2).broadcast_to([128, 4, Tb, BCORE]))
                ssl = sblk[:, :4 * Nk].rearrange("p (c t b) -> p c t b", c=4, t=Tb)
                rsl = rt[:, :4 * Nk].rearrange("p (c t b) -> p c t b", c=4, t=Tb)
                if k == 0:
                    # chunked so the first compares pipeline with the DMA
                    for c in range(4):
                        nc.vector.tensor_tensor(
                            out=ssl[:, c:c + 1], in0=xc[:, c:c + 1],
                            in1=rsl[:, c:c + 1], op=A.is_gt)
                else:
                    nc.vector.tensor_tensor(out=ssl, in0=xc, in1=rsl, op=A.is_gt)
                sblk_t[k] = sblk

            def stage_mm0(k):
                # H0 = S @ W0 -> drive tile k, slot lanes [0:256), t-major
                Tb = BLOCK_SIZES[k]
                Nk = Tb * BCORE
                sblk = sblk_t.pop(k)
                w0t = w0h_sb
                drv = drv_t[k]
                dv = drv[:].rearrange("p (t l) -> p t l", t=TBM)
                for c in range(8):
                    ps = ps_pool.tile([128, TBM * BCORE], f32, tag="ps")
                    for ki in range(4):
                        nc.tensor.matmul(
                            ps[:, :Nk],
                            lhsT=w0t[:, ki * 1024 + c * 128: ki * 1024 + (c + 1) * 128],
                            rhs=sblk[:, ki * Nk:(ki + 1) * Nk],
                            start=(ki == 0), stop=(ki == 3))
                    # PSUM (t,b) -> drive slots, bias fold
                    nc.scalar.activation(
                        out=dv[:, 0:Tb, c * BCORE:(c + 1) * BCORE],
                        in_=ps[:, :Nk].rearrange("p (t b) -> p t b", t=Tb),
                        func=Act.Identity, bias=b0_sb[:, c:c + 1], scale=1.0)

            def stage_mm1(k):
                # H1 = strace @ W1 -> drive tile k+2, slot lanes [256:512)
                Tb = BLOCK_SIZES[k]
                Nk = Tb * BCORE
                s0blk = s0_t[k]
                # slots are 512 wide: lanes [0:256) trace, [256:512) abar
                s0v = s0blk[:, :Tb * 512].rearrange("p (t l) -> p t l", t=Tb)
                drv = drv_t[k + 2]
                dv = drv[:].rearrange("p (t l) -> p t l", t=TBM)
                for c in range(8):
                    ps = ps_pool.tile([128, TBM * BCORE], f32, tag="ps")
                    for ki in range(8):
                        nc.tensor.matmul(
                            ps[:, :Nk],
                            lhsT=w1_sb[:, ki * 1024 + c * 128: ki * 1024 + (c + 1) * 128],
                            rhs=s0v[:, :, ki * BCORE:(ki + 1) * BCORE],
                            start=(ki == 0), stop=(ki == 7))
                    nc.scalar.activation(
                        out=dv[:, 0:Tb, 256 + c * BCORE:256 + (c + 1) * BCORE],
                        in_=ps[:, :Nk].rearrange("p (t b) -> p t b", t=Tb),
                        func=Act.Copy)

            def steps(k):
                """Per-step fused recurrences for iteration k:
                L0 on block k (if k < NB), L1 on block k-2 (if k >= 2)."""
                l0 = k if k < NB else None
                l1 = k - 2 if k >= 2 else None
                n0 = BLOCK_SIZES[l0] if l0 is not None else 0
                n1 = BLOCK_SIZES[l1] if l1 is not None else 0
                drv = drv_t[k]
                if l0 is not None:
                    s0blk = s0_pool.tile([128, TBM * 512], f16, tag="s0")
                    prev_blk = s0_t.get(l0 - 1)
                    s0_t[l0] = s0blk
                for t in range(max(n0, n1)):
                    do0 = l0 is not None and t < n0
                    do1 = l1 is not None and t < n1
                    p = gstep[0] % 2
                    gstep[0] += 1
                    src, dst = nm[p], nm[1 - p]
                    slot = drv[:, t * 512:(t + 1) * 512]
                    if do0 and do1:
                        nc.vector._custom_dve(
                            OP_RESET, out=dst[:], in0=src[:],
                            in1=slot, s0=-BETA, s1=THR)
                    elif do0:
                        nc.vector._custom_dve(
                            OP_RESET, out=dst[:, 0:256], in0=src[:, 0:256],
                            in1=slot[:, 0:256], s0=-BETA, s1=THR)
                    elif do1:
                        nc.vector._custom_dve(
                            OP_RESET, out=dst[:, 256:512], in0=src[:, 256:512],
                            in1=slot[:, 256:512], s0=-BETA, s1=THR)
                    if do0 and do1:
                        # fused trace-EMA + weighted-spike accumulation:
                        # [trace | abar] ride the 512-wide fp16 slots
                        if t > 0:
                            tp = s0blk[:, (t - 1) * 512:t * 512]
                        else:
                            pt = BLOCK_SIZES[l0 - 1] - 1
                            tp = prev_blk[:, pt * 512:(pt + 1) * 512]
                        nc.vector._custom_dve(
                            OP_TA,
                            out=s0blk[:, t * 512:(t + 1) * 512].rearrange(
                                "p (s l) -> p s l", s=2),
                            in0=dst[:, 0:512].rearrange("p (s l) -> p s l", s=2),
                            in1=tp.rearrange("p (s l) -> p s l", s=2),
                            s0=ALPHA, s1=float(W[TSTART[l1] + t]))
                    elif do1:
                        # abar-only step: hand the running value off from
                        # the last combined slot to the fp32 abar tile
                        ab = abar2 if l1 == NB - 1 else abar
                        if (l0 is not None and t == n0 and ab is abar):
                            ab_in = s0blk[:, (t - 1) * 512 + 256:t * 512]
                        else:
                            ab_in = ab[:]
                        nc.vector._custom_dve(
                            OP_ABAR, out=ab[:], in0=dst[:, 256:512],
                            in1=ab_in, s0=float(W[TSTART[l1] + t]))
                    elif do0:
                        # trace-only step (layer-1 not yet in flight)
                        tslot = s0blk[:, t * 512:t * 512 + 256]
                        if t > 0:
                            tprev = s0blk[:, (t - 1) * 512:(t - 1) * 512 + 256]
                        elif prev_blk is not None:
                            pt = BLOCK_SIZES[l0 - 1] - 1
                            tprev = prev_blk[:, pt * 512:pt * 512 + 256]
                        else:
                            tprev = None
                        if tprev is None:
                            nc.vector.tensor_scalar(
                                out=tslot, in0=dst[:, 0:256], scalar1=0.0,
                                scalar2=None, op0=A.is_equal)
                        else:
                            nc.vector._custom_dve(
                                OP_TRACE, out=tslot, in0=dst[:, 0:256],
                                in1=tprev, s0=ALPHA)

            # ---------------- schedule ----------------
            # rt(0) + fp16 W0 first: they gate the first mm0
            stage_dma_rt(0)
            nc.sync.dma_start(
                out=w0h_sb[:].rearrange("p (k m) -> p k m", k=4),
                in_=W0h.rearrange("(k p) m -> p k m", p=128))
            stage_dma_rt(1)
            stage_dma_rt(2)
            stage_sg(0)
            nc.sync.dma_start(
                out=w1_sb[:].rearrange("p (k m) -> p k m", k=8),
                in_=W1d.rearrange("(k p) m -> p k m", p=128))
            stage_sg(1)
            drv_t[0] = drv_pool.tile([128, 512 * TBM], f32, tag="drv",
                                     name="drv0")
            stage_mm0(0)

            for k in range(NB + 2):
                if k + 3 < NB:
                    stage_dma_rt(k + 3)
                # drive tile for iteration k+1 gets h1(k-1) and h0(k+1)
                if k + 1 <= NB + 1:
                    drv_t[k + 1] = drv_pool.tile(
                        [128, 512 * TBM], f32, tag="drv", name=f"drv{k + 1}")
                if 1 <= k <= NB:
                    stage_mm1(k - 1)
                if k + 1 < NB:
                    stage_mm0(k + 1)
                if k == NB - 1:
                    # W2 (fp16) arrives late, into a freed spike-block buffer
                    w2_sb = s_pool.tile([128, 8 * 512], f16, tag="sblk",
                                        name="w2_sb")
                    nc.sync.dma_start(
                        out=w2_sb[:].rearrange("p (k m) -> p k m", k=8),
                        in_=W2d.rearrange("(k p) m -> p k m", p=128))
                # abar-in-slot chain stitches at block-size mismatches:
                if k == 2:
                    # zero the abar lanes the first combined step will read
                    ls = BLOCK_SIZES[1] - 1
                    nc.vector.memset(
                        s0_t[1][:, ls * 512 + 256:(ls + 1) * 512], 0.0)
                if 3 <= k <= NB - 1 and k - 3 >= 0:
                    # stitch the abar chain across iteration k-1's ragged
                    # tail into the slot iteration k's t=0 will read
                    n0p, n1p = BLOCK_SIZES[k - 1], BLOCK_SIZES[k - 3]
                    if n0p > n1p:
                        # chain stopped early in slot n1p-1 (L0-only tail)
                        nc.vector.tensor_copy(
                            s0_t[k - 1][:, (n0p - 1) * 512 + 256:n0p * 512],
                            s0_t[k - 1][:, (n1p - 1) * 512 + 256:n1p * 512])
                    elif n1p > n0p:
                        # chain handed off to the fp32 tile (L1-only tail)
                        nc.vector.tensor_copy(
                            s0_t[k - 1][:, (n0p - 1) * 512 + 256:n0p * 512],
                            abar[:])
                steps(k)
                if k == NB:
                    # all of abar except the last L1 block is final: start
                    # mem2 = abar @ W2 in PSUM while the drain steps run
                    af = cpool.tile([128, 256], f16, tag="af")
                    nc.vector.tensor_copy(af[:], abar[:])
                    psf = ps_pool.tile([BCORE, 512], f32, tag="psf")
                    for ki in range(8):
                        nc.tensor.matmul(
                            psf[:],
                            lhsT=af[:, ki * BCORE:(ki + 1) * BCORE],
                            rhs=w2_sb[:, ki * 512:(ki + 1) * 512],
                            start=(ki == 0), stop=False)
                if k + 2 < NB:
                    stage_sg(k + 2)

            # ---- final: mem2 += abar2 @ W2 (PSUM accumulate) ----
            af2 = cpool.tile([128, 256], f16, tag="af2")
            nc.vector.tensor_copy(af2[:], abar2[:])
            for ki in range(8):
                nc.tensor.matmul(
                    psf[:],
                    lhsT=af2[:, ki * BCORE:(ki + 1) * BCORE],
                    rhs=w2_sb[:, ki * 512:(ki + 1) * 512],
                    start=False, stop=(ki == 7))
            outsb = cpool.tile([BCORE, 512], f32, tag="outsb")
            nc.scalar.activation(out=outsb[:], in_=psf[:], func=Act.Copy)
            nc.sync.dma_start(out=outd, in_=outsb[:])

    nc.compile()
    _CACHE["nc"] = nc
    return nc


def make_in_maps(inputs, W0, W1, W2, random_distribution):
    inputs = np.ascontiguousarray(np.asarray(inputs, np.float32))
    W0 = np.asarray(W0, np.float32)
    W1 = np.asarray(W1, np.float32)
    W2 = np.asarray(W2, np.float32)
    R = np.asarray(random_distribution, np.float32)

    W0h16 = np.ascontiguousarray(W0[:512].astype(np.float16))
    W1r = np.ascontiguousarray(W1.astype(np.float16))
    W2r = np.ascontiguousarray(W2.astype(np.float16))
    b0 = np.ascontiguousarray(W0[512].reshape(8, 128).T)  # [128, 8]

    in_maps = []
    for i in range(NCORES):
        sl = slice(i * BCORE, (i + 1) * BCORE)
        xTi = np.ascontiguousarray(inputs[sl].T)  # [512, 32]
        RTi = np.ascontiguousarray(
            R[1:, sl, :512].transpose(2, 0, 1).reshape(512, T * BCORE))
        in_maps.append({
            "RT": RTi, "xT": xTi, "W0h": W0h16,
            "W1d": W1r, "W2d": W2r, "b0d": b0,
        })
    return in_maps


def kernel(inputs, W0, W1, W2, random_distribution):
    from concourse.bass_utils import run_bass_kernel_spmd
    nc = build_program()
    in_maps = make_in_maps(inputs, W0, W1, W2, random_distribution)
    res = run_bass_kernel_spmd(nc, in_maps, core_ids=list(range(NCORES)))
    outs = [np.asarray(res.results[i]["out"], np.float32) for i in range(NCORES)]
    return np.concatenate(outs, axis=0)


if __name__ == "__main__":
    d = np.load("/tmp/snn_inputs.npz")
    out = kernel(d["inputs"], d["W0"], d["W1"], d["W2"], d["random_distribution"])
    exp = d["expected"]
    rel = np.linalg.norm(out - exp) / np.linalg.norm(exp)
    print("kernel vs reference rel_l2:", rel)


# revision 76
# speedup vs baseline: 1.0804x; 1.0804x over previous
"""Trainium2 Bass kernel for the 3-layer spiking neural network (DSNN).

Strategy
--------
Data-parallel over batch: 256 rows / 8 cores = 32 per core, weights
replicated, zero collectives. Inside each core:

  - Layer 2 has no reset, so mem2 = (sum_t w_t out1(t)) @ W2 exactly
    (closed-form alpha/beta decay weights): one small final matmul,
    split into two PSUM-accumulated halves so most of it runs during
    the pipeline-drain iterations.
  - The layer-1 synapse recurrence is folded into the matmul operand:
    mm1's moving tensor is the spike TRACE  strace_t = a*strace_{t-1} + s0_t,
    so  y1_t = strace_t @ W1  exactly (linearity) - no separate y1
    state, no per-step AXPY. The trace is produced by one fused DVE op
    per step (SNN_TRACE) straight into the fp16 matmul-operand slot.
  - Timestep-blocked matmuls (Tb=16 -> 512 moving columns). All
    weights, spikes and the trace run in fp16: halves LDWEIGHTS to
    ~135ns (the f32r LDW at 242ns was the PE rate limiter), streams
    at 1 cyc/row at any FD, and drops the f32r W0 tile + its DMA.
    Measured rel-l2 1.295e-2 vs the 2e-2 gate; the host-side
    bit-accurate simulator predicted 1.2946e-2 (it has matched HW to
    ~1e-4 on every variant tried).
  - Membrane recurrences of layer 0 (block k) and layer 1 (block k-2)
    run as ONE fused custom DVE op per step (SNN_RESET on [128,512],
    t-major contiguous drive slots [h0_t | h1_t] filled directly by
    the ScalarE PSUM drains of both matmuls): state is the negated
    membrane; 0.0 encodes "spiked". Layer-1 trails two blocks so its
    drive h1 = trace @ W1 is ready. Spike * w_t accumulation for the
    collapsed layer 2 is one more fused op (SNN_ABAR).
  - Membrane state ping-pongs between two tiles (nmA/nmB) so readers
    never stall the serial reset chain.

Per main step the DVE runs exactly TWO fused ops: SNN_RESET [128,512]
(both layers' membranes) and SNN_TA [128,2,256] (a SubIdx-paged op
whose page 0 is the trace EMA and page 1 the weighted-spike abar
accumulation, riding the 512-wide fp16 slots; ragged block boundaries
hand the running abar off to an fp32 tile). Measured step period
~1.33us (down from 2.65us/step in the original); PE runs 3072 fp16
moving columns well under that; ScalarE drains PSUM into the drive
slots. Measured: 190.0us (baseline 273.6us).
"""

import numpy as np

ALPHA = 0.9
BETA = 0.85
THR = 1.0
T = 99            # timesteps actually simulated (t = 1..99 of 100)
BCORE = 32        # batch per core
NCORES = 8
TBM = 16          # main block size (Nk = 512 moving cols)
BLOCK_SIZES = [8, 16, 16, 16, 16, 16, 8, 3]
assert sum(BLOCK_SIZES) == T
NB = len(BLOCK_SIZES)
TSTART = [sum(BLOCK_SIZES[:i]) for i in range(NB)]

_CACHE = {}


def _register_custom_ops():
    """SNN_RESET: m = in0*s0 + in1; out = m>s1 ? 0 : -m   (negated membrane;
    0.0 encodes "spiked").
    SNN_TRACE: out = in1*s0 + (in0 == 0)                  (spike trace EMA).
    SNN_ABAR:  out = (in0 == 0) ? in1 + s0 : in1          (weighted spikes).
    """
    import concourse.dve_ops as dve_ops
    if "SNN_RESET" in dve_ops._SUB_OPCODE_FOR_NAME:
        return (next(o for o in dve_ops.OPS if o.name == "SNN_RESET"),
                next(o for o in dve_ops.OPS if o.name == "SNN_TRACE"),
                next(o for o in dve_ops.OPS if o.name == "SNN_ABAR"),
                next(o for o in dve_ops.OPS if o.name == "SNN_TA"))
    from concourse.dve_spec import (
        Spec, Src0, Src1, Zero, select, eq, lower, _has_src1, SubIdx)
    from concourse.dve_uop import DveOpSpec

    def make(name, spec, subdim=False):
        row = dve_ops._CUSTOM_DVE_ROW_BASE + len(dve_ops.OPS)
        assert row < 0x20
        dve_ops._SUB_OPCODE_FOR_NAME[name] = row
        shas = {}
        for ver in ("v3", "v4"):
            uops = lower(spec, ver=ver)
            shas[ver] = DveOpSpec(name=name, opcode=row, uops=uops,
                                  rd1_en=_has_src1(spec)).sha(ver)
        op = dve_ops.DveOp(name, spec, subdim=subdim, uops_sha=shas)
        dve_ops.OPS.append(op)
        dve_ops.CUSTOM_DVE_SPECS[name] = spec
        return op

    from concourse.dve_spec import C0, C1
    f32 = np.float32
    _m = Src0 * C0 + Src1
    reset = make("SNN_RESET", Spec(
        body=select(_m > C1, Zero, Zero - _m),
        reference=lambda in0, in1, s0, s1, imm2:
            np.where((in0 * f32(s0) + in1) > f32(s1),
                     f32(0.0), -(in0 * f32(s0) + in1)).astype(f32),
    ))
    trace = make("SNN_TRACE", Spec(
        body=Src1 * C0 + eq(Src0, Zero),
        reference=lambda in0, in1, s0, s1, imm2:
            (in1 * f32(s0) + (in0 == 0.0)).astype(f32),
    ))
    abar_op = make("SNN_ABAR", Spec(
        body=select(eq(Src0, Zero), Src1 + C0, Src1),
        reference=lambda in0, in1, s0, s1, imm2:
            np.where(in0 == 0.0, in1 + f32(s0), in1).astype(f32),
    ))

    def _ta_ref(in0, in1, s0, s1, imm2):
        # in0/in1/out: [P, 2, N] pages; page 0 = trace, page 1 = abar
        sp = (in0 == 0.0).astype(f32)
        out = np.empty_like(in1, dtype=f32)
        out[:, 0] = in1[:, 0] * f32(s0) + sp[:, 0]
        out[:, 1] = np.where(in0[:, 1] == 0.0, in1[:, 1] + f32(s1), in1[:, 1])
        return out

    _sp = eq(Src0, Zero)
    ta_op = make("SNN_TA", Spec(
        body=select(SubIdx, Src1 + _sp * C1, Src1 * C0 + _sp),
        reference=_ta_ref,
    ), subdim=True)
    return reset, trace, abar_op, ta_op


def _round_m11(x):
    # hw float32r = e8m11, round-to-nearest on the 12 dropped bits
    xi = np.ascontiguousarray(np.asarray(x, np.float32)).view(np.uint32).astype(np.uint64)
    bias = np.uint64(0x7FF) + ((xi >> np.uint64(12)) & np.uint64(1))
    return ((xi + bias) & np.uint64(0xFFFFF000)).astype(np.uint32).view(np.float32)


def _decay_weights():
    # w_j = sum_{k=0}^{T-1-j} BETA^(T-1-j-k) * ALPHA^k
    w = np.zeros(T, np.float64)
    for j in range(T):
        n = T - 1 - j
        k = np.arange(n + 1)
        w[j] = np.sum(BETA ** (n - k) * (ALPHA ** k))
    return w.astype(np.float32)


def build_program():
    if "nc" in _CACHE:
        return _CACHE["nc"]
    import concourse.bacc as bacc
    import concourse.mybir as mybir
    import concourse.tile as tile

    f32 = mybir.dt.float32
    f32r = mybir.dt.float32r
    f16 = mybir.dt.float16
    A = mybir.AluOpType
    Act = mybir.ActivationFunctionType

    OP_RESET, OP_TRACE, OP_ABAR, OP_TA = _register_custom_ops()
    W = _decay_weights()

    nc = bacc.Bacc("TRN2", target_bir_lowering=False, debug=False,
                   enable_asserts=False, num_devices=NCORES)

    RT = nc.dram_tensor("RT", [512, T * BCORE], f32, kind="ExternalInput").ap()
    xT = nc.dram_tensor("xT", [512, BCORE], f32, kind="ExternalInput").ap()
    W0h = nc.dram_tensor("W0h", [512, 1024], f16, kind="ExternalInput").ap()
    W1d = nc.dram_tensor("W1d", [1024, 1024], f16, kind="ExternalInput").ap()
    W2d = nc.dram_tensor("W2d", [1024, 512], f16, kind="ExternalInput").ap()
    b0d = nc.dram_tensor("b0d", [128, 8], f32, kind="ExternalInput").ap()
    outd = nc.dram_tensor("out", [BCORE, 512], f32, kind="ExternalOutput").ap()

    with tile.TileContext(nc) as tc:
        with (
            tc.tile_pool(name="const", bufs=1) as cpool,
            tc.tile_pool(name="rt", bufs=3) as rt_pool,
            tc.tile_pool(name="sblk", bufs=2) as s_pool,
            tc.tile_pool(name="s0p", bufs=2) as s0_pool,
            tc.tile_pool(name="drv", bufs=2) as drv_pool,
            tc.tile_pool(name="ps", bufs=4, space="PSUM") as ps_pool,
        ):
            # ---- constants ----
            w0h_sb = cpool.tile([128, 4 * 1024], f16, tag="w0h")
            w1_sb = cpool.tile([128, 8 * 1024], f16, tag="w1")
            b0_sb = cpool.tile([128, 8], f32, tag="b0")
            xt_sb = cpool.tile([128, 4 * BCORE], f32, tag="xt")

            nc.sync.dma_start(
                out=xt_sb[:].rearrange("p (c b) -> p c b", c=4),
                in_=xT.rearrange("(c p) b -> p c b", p=128))
            nc.sync.dma_start(out=b0_sb[:], in_=b0d)

            # ---- state ----
            # negm ping-pong: [0:256) = layer-0 negm (c,b), [256:512) = layer-1
            nmA = cpool.tile([128, 512], f32, tag="nmA")
            nmB = cpool.tile([128, 512], f32, tag="nmB")
            abar = cpool.tile([128, 256], f32, tag="abar")
            abar2 = cpool.tile([128, 256], f32, tag="abar2")
            for st in (nmA, nmB, abar, abar2):
                nc.vector.memset(st[:], 0.0)
            nm = [nmA, nmB]
            gstep = [0]

            rt4 = RT.rearrange("(c p) n -> p c n", p=128)
            rt_t, sblk_t, s0_t, drv_t = {}, {}, {}, {}

            def stage_dma_rt(k):
                Tb = BLOCK_SIZES[k]
                Nk = Tb * BCORE
                rt = rt_pool.tile([128, 4 * TBM * BCORE], f32, tag="rt")
                for c in range(4):
                    nc.sync.dma_start(
                        out=rt[:, c * Nk:(c + 1) * Nk],
                        in_=rt4[:, c, TSTART[k] * BCORE: TSTART[k] * BCORE + Nk])
                rt_t[k] = rt

            def stage_sg(k):
                # spike-gen: compare x (broadcast over t) against rt.
                # Output dtype matches the W0 flavor mm0 will use:
                # f32r for early blocks, fp16 for the small late blocks.
                Tb = BLOCK_SIZES[k]
                Nk = Tb * BCORE
                rt = rt_t.pop(k)
                sblk = s_pool.tile([128, 4 * TBM * BCORE], f16, tag="sblk")
                xc = (xt_sb[:].rearrange("p (c b) -> p c b", c=4)
                      .unsqueeze(2).broadcast_to([128, 4, Tb, BCORE]))
                ssl = sblk[:, :4 * Nk].rearrange("p (c t b) -> p c t b", c=4, t=Tb)
                rsl = rt[:, :4 * Nk].rearrange("p (c t b) -> p c t b", c=4, t=Tb)
                if k == 0:
                    # chunked so the first compares pipeline with the DMA
                    for c in range(4):
                        nc.vector.tensor_tensor(
                            out=ssl[:, c:c + 1], in0=xc[:, c:c + 1],
                            in1=rsl[:, c:c + 1], op=A.is_gt)
                else:
                    nc.vector.tensor_tensor(out=ssl, in0=xc, in1=rsl, op=A.is_gt)
                sblk_t[k] = sblk

            def stage_mm0(k):
                # H0 = S @ W0 -> drive tile k, slot lanes [0:256), t-major
                Tb = BLOCK_SIZES[k]
                Nk = Tb * BCORE
                sblk = sblk_t.pop(k)
                w0t = w0h_sb
                drv = drv_t[k]
                dv = drv[:].rearrange("p (t l) -> p t l", t=TBM)
                for c in range(8):
                    ps = ps_pool.tile([128, TBM * BCORE], f32, tag="ps")
                    for ki in range(4):
                        nc.tensor.matmul(
                            ps[:, :Nk],
                            lhsT=w0t[:, ki * 1024 + c * 128: ki * 1024 + (c + 1) * 128],
                            rhs=sblk[:, ki * Nk:(ki + 1) * Nk],
                            start=(ki == 0), stop=(ki == 3))
                    # PSUM (t,b) -> drive slots, bias fold
                    nc.scalar.activation(
                        out=dv[:, 0:Tb, c * BCORE:(c + 1) * BCORE],
                        in_=ps[:, :Nk].rearrange("p (t b) -> p t b", t=Tb),
                        func=Act.Identity, bias=b0_sb[:, c:c + 1], scale=1.0)

            def stage_mm1(k):
                # H1 = strace @ W1 -> drive tile k+2, slot lanes [256:512)
                Tb = BLOCK_SIZES[k]
                Nk = Tb * BCORE
                s0blk = s0_t[k]
                # slots are 512 wide: lanes [0:256) trace, [256:512) abar
                s0v = s0blk[:, :Tb * 512].rearrange("p (t l) -> p t l", t=Tb)
                drv = drv_t[k + 2]
                dv = drv[:].rearrange("p (t l) -> p t l", t=TBM)
                for c in range(8):
                    ps = ps_pool.tile([128, TBM * BCORE], f32, tag="ps")
                    for ki in range(8):
                        nc.tensor.matmul(
                            ps[:, :Nk],
                            lhsT=w1_sb[:, ki * 1024 + c * 128: ki * 1024 + (c + 1) * 128],
                            rhs=s0v[:, :, ki * BCORE:(ki + 1) * BCORE],
                            start=(ki == 0), stop=(ki == 7))
                    nc.scalar.activation(
                        out=dv[:, 0:Tb, 256 + c * BCORE:256 + (c + 1) * BCORE],
                        in_=ps[:, :Nk].rearrange("p (t b) -> p t b", t=Tb),
                        func=Act.Copy)

            def steps(k):
                """Per-step fused recurrences for iteration k:
                L0 on block k (if k < NB), L1 on block k-2 (if k >= 2)."""
                l0 = k if k < NB else None
                l1 = k - 2 if k >= 2 else None
                n0 = BLOCK_SIZES[l0] if l0 is not None else 0
                n1 = BLOCK_SIZES[l1] if l1 is not None else 0
                drv = drv_t[k]
                if l0 is not None:
                    s0blk = s0_pool.tile([128, TBM * 512], f16, tag="s0")
                    prev_blk = s0_t.get(l0 - 1)
                    s0_t[l0] = s0blk
                for t in range(max(n0, n1)):
                    do0 = l0 is not None and t < n0
                    do1 = l1 is not None and t < n1
                    p = gstep[0] % 2
                    gstep[0] += 1
                    src, dst = nm[p], nm[1 - p]
                    slot = drv[:, t * 512:(t + 1) * 512]
                    if do0 and do1:
                        nc.vector._custom_dve(
                            OP_RESET, out=dst[:], in0=src[:],
                            in1=slot, s0=-BETA, s1=THR)
                    elif do0:
                        nc.vector._custom_dve(
                            OP_RESET, out=dst[:, 0:256], in0=src[:, 0:256],
                            in1=slot[:, 0:256], s0=-BETA, s1=THR)
                    elif do1:
                        nc.vector._custom_dve(
                            OP_RESET, out=dst[:, 256:512], in0=src[:, 256:512],
                            in1=slot[:, 256:512], s0=-BETA, s1=THR)
                    if do0 and do1:
                        # fused trace-EMA + weighted-spike accumulation:
                        # [trace | abar] ride the 512-wide fp16 slots
                        if t > 0:
                            tp = s0blk[:, (t - 1) * 512:t * 512]
                        else:
                            pt = BLOCK_SIZES[l0 - 1] - 1
                            tp = prev_blk[:, pt * 512:(pt + 1) * 512]
                        nc.vector._custom_dve(
                            OP_TA,
                            out=s0blk[:, t * 512:(t + 1) * 512].rearrange(
                                "p (s l) -> p s l", s=2),
                            in0=dst[:, 0:512].rearrange("p (s l) -> p s l", s=2),
                            in1=tp.rearrange("p (s l) -> p s l", s=2),
                            s0=ALPHA, s1=float(W[TSTART[l1] + t]))
                    elif do1:
                        # abar-only step: hand the running value off from
                        # the last combined slot to the fp32 abar tile
                        ab = abar2 if l1 == NB - 1 else abar
                        if (l0 is not None and t == n0 and ab is abar):
                            ab_in = s0blk[:, (t - 1) * 512 + 256:t * 512]
                        else:
                            ab_in = ab[:]
                        nc.vector._custom_dve(
                            OP_ABAR, out=ab[:], in0=dst[:, 256:512],
                            in1=ab_in, s0=float(W[TSTART[l1] + t]))
                    elif do0:
                        # trace-only step (layer-1 not yet in flight)
                        tslot = s0blk[:, t * 512:t * 512 + 256]
                        if t > 0:
                            tprev = s0blk[:, (t - 1) * 512:(t - 1) * 512 + 256]
                        elif prev_blk is not None:
                            pt = BLOCK_SIZES[l0 - 1] - 1
                            tprev = prev_blk[:, pt * 512:pt * 512 + 256]
                        else:
                            tprev = None
                        if tprev is None:
                            nc.vector.tensor_scalar(
                                out=tslot, in0=dst[:, 0:256], scalar1=0.0,
                                scalar2=None, op0=A.is_equal)
                        else:
                            nc.vector._custom_dve(
                                OP_TRACE, out=tslot, in0=dst[:, 0:256],
                                in1=tprev, s0=ALPHA)

            # ---------------- schedule ----------------
            # rt(0) + fp16 W0 first: they gate the first mm0
            stage_dma_rt(0)
            nc.sync.dma_start(
                out=w0h_sb[:].rearrange("p (k m) -> p k m", k=4),
                in_=W0h.rearrange("(k p) m -> p k m", p=128))
            stage_dma_rt(1)
            stage_dma_rt(2)
            stage_sg(0)
            nc.sync.dma_start(
                out=w1_sb[:].rearrange("p (k m) -> p k m", k=8),
                in_=W1d.rearrange("(k p) m -> p k m", p=128))
            stage_sg(1)
            drv_t[0] = drv_pool.tile([128, 512 * TBM], f32, tag="drv",
                                     name="drv0")
            stage_mm0(0)

            for k in range(NB + 2):
                if k + 3 < NB:
                    stage_dma_rt(k + 3)
                # drive tile for iteration k+1 gets h1(k-1) and h0(k+1)
                if k + 1 <= NB + 1:
                    drv_t[k + 1] = drv_pool.tile(
                        [128, 512 * TBM], f32, tag="drv", name=f"drv{k + 1}")
                if 1 <= k <= NB:
                    stage_mm1(k - 1)
                if k + 1 < NB:
                    stage_mm0(k + 1)
                if k == NB - 1:
                    # W2 (fp16) arrives late, into a freed spike-block buffer
                    w2_sb = s_pool.tile([128, 8 * 512], f16, tag="sblk",
                                        name="w2_sb")
                    nc.sync.dma_start(
                        out=w2_sb[:].rearrange("p (k m) -> p k m", k=8),
                        in_=W2d.rearrange("(k p) m -> p k m", p=128))
                # abar-in-slot chain stitches at block-size mismatches:
                if k == 2:
                    # zero the abar lanes the first combined step will read
                    ls = BLOCK_SIZES[1] - 1
                    nc.vector.memset(
                        s0_t[1][:, ls * 512 + 256:(ls + 1) * 512], 0.0)
                if k == 3 and BLOCK_SIZES[0] < BLOCK_SIZES[2]:
                    # iter-2's combined phase ended at slot n1-1; move the
                    # running abar to the slot iter-3's t=0 will read
                    sa = BLOCK_SIZES[0] - 1
                    da = BLOCK_SIZES[2] - 1
                    nc.vector.tensor_copy(
                        s0_t[2][:, da * 512 + 256:(da + 1) * 512],
                        s0_t[2][:, sa * 512 + 256:(sa + 1) * 512])
                if k == NB - 1:
                    # iter-6 finished abar on the fp32 tile; seed it back
                    # into the slot iter-7's combined t=0 will read
                    ls = BLOCK_SIZES[NB - 2] - 1
                    nc.vector.tensor_copy(
                        s0_t[NB - 2][:, ls * 512 + 256:(ls + 1) * 512],
                        abar[:])
                steps(k)
                if k == NB:
                    # all of abar except the last L1 block is final: start
                    # mem2 = abar @ W2 in PSUM while the drain steps run
                    af = cpool.tile([128, 256], f16, tag="af")
                    nc.vector.tensor_copy(af[:], abar[:])
                    psf = ps_pool.tile([BCORE, 512], f32, tag="psf")
                    for ki in range(8):
                        nc.tensor.matmul(
                            psf[:],
                            lhsT=af[:, ki * BCORE:(ki + 1) * BCORE],
                            rhs=w2_sb[:, ki * 512:(ki + 1) * 512],
                            start=(ki == 0), stop=False)
                if k + 2 < NB:
                    stage_sg(k + 2)

            # ---- final: mem2 += abar2 @ W2 (PSUM accumulate) ----
            af2 = cpool.tile([128, 256], f16, tag="af2")
            nc.vector.tensor_copy(af2[:], abar2[:])
            for ki in range(8):
                nc.tensor.matmul(
                    psf[:],
                    lhsT=af2[:, ki * BCORE:(ki + 1) * BCORE],
                    rhs=w2_sb[:, ki * 512:(ki + 1) * 512],
                    start=False, stop=(ki == 7))
            outsb = cpool.tile([BCORE, 512], f32, tag="outsb")
            nc.scalar.activation(out=outsb[:], in_=psf[:], func=Act.Copy)
            nc.sync.dma_start(out=outd, in_=outsb[:])

    nc.compile()
    _CACHE["nc"] = nc
    return nc


def make_in_maps(inputs, W0, W1, W2, random_distribution):
    inputs = np.ascontiguousarray(np.asarray(inputs, np.float32))
    W0 = np.asarray(W0, np.float32)
    W1 = np.asarray(W1, np.float32)
    W2 = np.asarray(W2, np.float32)
    R = np.asarray(random_distribution, np.float32)

    W0h16 = np.ascontiguousarray(W0[:512].astype(np.float16))
    W1r = np.ascontiguousarray(W1.astype(np.float16))
    W2r = np.ascontiguousarray(W2.astype(np.float16))
    b0 = np.ascontiguousarray(W0[512].reshape(8, 128).T)  # [128, 8]

    in_maps = []
    for i in range(NCORES):
        sl = slice(i * BCORE, (i + 1) * BCORE)
        xTi = np.ascontiguousarray(inputs[sl].T)  # [512, 32]
        RTi = np.ascontiguousarray(
            R[1:, sl, :512].transpose(2, 0, 1).reshape(512, T * BCORE))
        in_maps.append({
            "RT": RTi, "xT": xTi, "W0h": W0h16,
            "W1d": W1r, "W2d": W2r, "b0d": b0,
        })
    return in_maps


def kernel(inputs, W0, W1, W2, random_distribution):
    from concourse.bass_utils import run_bass_kernel_spmd
    nc = build_program()
    in_maps = make_in_maps(inputs, W0, W1, W2, random_distribution)
    res = run_bass_kernel_spmd(nc, in_maps, core_ids=list(range(NCORES)))
    outs = [np.asarray(res.results[i]["out"], np.float32) for i in range(NCORES)]
    return np.concatenate(outs, axis=0)


if __name__ == "__main__":
    d = np.load("/tmp/snn_inputs.npz")
    out = kernel(d["inputs"], d["W0"], d["W1"], d["W2"], d["random_distribution"])
    exp = d["expected"]
    rel = np.linalg.norm(out - exp) / np.linalg.norm(exp)
    print("kernel vs reference rel_l2:", rel)
